# revision 26
# baseline (speedup 1.0000x reference)
"""BlurDownsample Trainium2 kernel.

Reference op: depthwise 3x3 binomial blur ([1,2,1] outer product / 16,
stride 1, zero padding 1) followed by exact 2x2 average-pool downsample.
Composed, this is a separable 4-tap stride-2 filter:

    o[i,j] = (1/64) * sum_{a,b in 0..3} w[a] w[b] x[2i-1+a, 2j-1+b],
    w = [1,3,3,1], taps outside [0,256) dropped (zero padding).

Input  x: (8, 128, 256, 256) f32  ->  output (8, 128, 128, 128) f32.

Sharding: pure data-parallel over batch. Core n handles x[n].

Per-core pipeline (128 channel planes, groups of GP=8 planes):
  1. One DMA per group: xt[p, c, (e w)] = x[c, 2p+e, w]. Partition p holds
     input row-pair (2p, 2p+1), so each partition's HBM source is one
     contiguous 2KB run -> efficient descriptors.
  2. Vertical pass on TensorE: T2[i] = sum_u Mv[i,u] x[u]. Contraction over
     partitions, split by row parity: lhsT_e[p, i] = Mv[2p+e, i] with
     integer weights {1,3,3,1}; two accumulating matmuls per PSUM region.
     Data is float32r (1 cycle/row vs 4 for full fp32).
  3. ScalarE drains PSUM -> SBUF with scale 1/64 into a guarded layout
     (one zero column each side of every plane for the horizontal pad).
  4. Horizontal pass: p = C[2j]+C[2j+1] (VectorE), q = C[2j-1]+C[2j+2]
     (GpSimdE), out = 3*p + q (VectorE fused scalar_tensor_tensor).
  5. DMA out on the scalar HWDGE ring (inputs use the sync ring; splitting
     the two rings measured ~25 us faster than sharing one).

Measured (reps-loop differencing, 8 cores in parallel): ~140 us/core;
cost-model timeline ~132 us; DMA-bytes floor (40 MB/core @ 358 GB/s)
~112 us. L2 relative error vs fp32 reference: 1.04e-4 (float32r matmul).
"""

import numpy as np

B, C, H, W = 8, 128, 256, 256
HO, WO = H // 2, W // 2
GP = 8            # channel planes per group
NG = C // GP      # groups per core
N_CORES = 8

_CACHE: dict = {}


def _mvt_weights() -> np.ndarray:
    """MVT[e][p, i] = vertical weight of input row 2p+e for output row i.

    Integer weights {1,3,3,1} at input rows 2i-1 .. 2i+2 (rows outside
    [0, 256) dropped -> zero padding). Normalization (1/64) is applied
    later on the ScalarE PSUM->SBUF copy.
    """
    m = np.zeros((H, HO), dtype=np.float32)
    w = (1.0, 3.0, 3.0, 1.0)
    for i in range(HO):
        for t in range(4):
            u = 2 * i - 1 + t
            if 0 <= u < H:
                m[u, i] = w[t]
    return np.ascontiguousarray(np.stack([m[0::2], m[1::2]], axis=0))


def _build(
    reps: int = 1,
    q_on_gpsimd: bool = True,
    out_on_scalar: bool = True,
    xbufs: int = 6,
    dma_only: bool = False,
    dma_alternate: bool = False,
    cbufs: int = 3,
    pqbufs: int = 2,
    obufs: int = 3,
    gp: int = GP,
    queue_mode: bool = False,
    static_ct: bool = False,
):
    import contextlib

    import concourse.bacc as bacc
    import concourse.mybir as mybir
    from concourse.tile import TileContext

    f32 = mybir.dt.float32
    f32r = mybir.dt.float32r
    COPY = mybir.ActivationFunctionType.Copy
    MULT = mybir.AluOpType.mult
    ADD = mybir.AluOpType.add

    nc = bacc.Bacc("TRN2", target_bir_lowering=False, debug=False)

    # xs/mvt are declared float32r (same 4-byte layout as f32) so the
    # TensorE matmul runs at 1 cycle/row instead of fp32's 4.
    xs = nc.dram_tensor("xs", [C, H, W], f32r, kind="ExternalInput")
    mvt = nc.dram_tensor("mvt", [2, 128, HO], f32r, kind="ExternalInput")
    out = nc.dram_tensor("out", [C, HO, WO], f32, kind="ExternalOutput")

    NGg = C // gp
    HGP_TILE = 4  # planes per PSUM tile (2 banks)
    HGP = HGP_TILE

    with TileContext(
        nc, pool_alloc_mode="queue" if queue_mode else "stack"
    ) as tc:
        with (
            tc.tile_pool(name="wpool", bufs=1) as wpool,
            tc.tile_pool(name="xpool", bufs=xbufs) as xpool,
            tc.tile_pool(name="psum", bufs=4, space="PSUM") as pspool,
            tc.tile_pool(name="cpool", bufs=cbufs) as cpool,
            tc.tile_pool(name="pqpool", bufs=pqbufs) as pqpool,
            tc.tile_pool(name="opool", bufs=obufs) as opool,
        ):
            # Stationary vertical filter, both row parities: wt[p, e, i]
            wt = wpool.tile([128, 2, HO], f32r)
            nc.sync.dma_start(out=wt[:], in_=mvt.rearrange("e p i -> p e i"))

            ct_slots = []
            if static_ct:
                # Persistent ct ring: guards zeroed once, reused g % cbufs.
                for si in range(cbufs):
                    cts = wpool.tile(
                        [128, gp, W + 2], f32, tag=f"ct{si}"
                    )
                    nc.gpsimd.memset(cts[:, :, 0 : W + 2 : W + 1], 0.0)
                    ct_slots.append(cts)

            loop_cm = (
                tc.For_i(
                    0,
                    reps,
                    1,
                    hint_engines=(
                        mybir.EngineType.SP,
                        mybir.EngineType.PE,
                        mybir.EngineType.DVE,
                        mybir.EngineType.Activation,
                        mybir.EngineType.Pool,
                    ),
                )
                if reps > 1
                else contextlib.nullcontext()
            )
            with loop_cm:
                for g in range(NGg):
                    c0 = g * gp

                    # xt[p, c, 512*e + w] = x[c0+c, 2p+e, w]
                    # One DMA, 2KB contiguous per (p, c) chunk.
                    if dma_alternate:
                        in_eng = nc.sync if g % 2 == 0 else nc.scalar
                        out_eng = nc.scalar if g % 2 == 0 else nc.sync
                    else:
                        in_eng = nc.sync
                        out_eng = nc.scalar if out_on_scalar else nc.sync
                    xt = xpool.tile([128, gp, 2 * W], f32r)
                    in_eng.dma_start(
                        out=xt[:],
                        in_=xs[c0 : c0 + gp]
                        .rearrange("c h w -> c (h w)")
                        .rearrange("c (p q) -> p c q", p=128),
                    )
                    xtv = xt.rearrange("p c (e w) -> p c e w", e=2)

                    if dma_only:
                        # Floor probe: ship input straight back out, no compute.
                        out_eng.dma_start(
                            out=out[c0 : c0 + gp].rearrange("c i j -> i c j"),
                            in_=xt[:, :, 0:WO].bitcast(f32),
                        )
                        continue

                    # Vertical pass: two PSUM tiles of 4 planes each; for
                    # each, accumulate even-row and odd-row contributions.
                    # ps[i, c, w] = sum_u Mv[i, u] x[c, u, w]
                    ct = ct_slots[g % cbufs] if static_ct else cpool.tile(
                        [128, gp, W + 2], f32
                    )
                    for half in range(gp // HGP_TILE):
                        ps = pspool.tile([128, HGP, W], f32, tag="ps")
                        cbase = half * HGP
                        for e in range(2):
                            for pp in range(HGP // 2):
                                nc.tensor.matmul(
                                    ps[:, 2 * pp : 2 * pp + 2, :],
                                    wt[:, e, :],
                                    xtv[:, cbase + 2 * pp : cbase + 2 * pp + 2, e, :],
                                    start=(e == 0),
                                    stop=(e == 1),
                                )
                        # Guarded copy: ct[i, c, 1+w] = ps[i, c, w] / 64
                        nc.scalar.activation(
                            ct[:, cbase : cbase + HGP, 1 : W + 1],
                            ps[:],
                            COPY,
                            scale=1.0 / 64.0,
                        )

                    if not static_ct:
                        # Zero guard columns (ct[..., 0] and ct[..., W+1]).
                        nc.gpsimd.memset(ct[:, :, 0 : W + 2 : W + 1], 0.0)

                    # Horizontal pass (col m of ct = combined col c_{m-1}):
                    #   p[j] = c_{2j}   + c_{2j+1} = ct[2j+1] + ct[2j+2]
                    #   q[j] = c_{2j-1} + c_{2j+2} = ct[2j]   + ct[2j+3]
                    #   o[j] = 3*p[j] + q[j]
                    pt = pqpool.tile([128, gp, WO], f32, tag="pt")
                    qt = pqpool.tile([128, gp, WO], f32, tag="qt")
                    nc.vector.tensor_add(
                        pt[:], ct[:, :, 1 : W + 1 : 2], ct[:, :, 2 : W + 2 : 2]
                    )
                    q_eng = nc.gpsimd if q_on_gpsimd else nc.vector
                    q_eng.tensor_add(
                        qt[:], ct[:, :, 0 : W : 2], ct[:, :, 3 : W + 2 : 2]
                    )
                    ot = opool.tile([128, gp, WO], f32)
                    nc.vector.scalar_tensor_tensor(
                        ot[:], pt[:], 3.0, qt[:], op0=MULT, op1=ADD
                    )

                    out_eng.dma_start(
                        out=out[c0 : c0 + gp].rearrange("c i j -> i c j"), in_=ot[:]
                    )

    nc.compile()
    return nc


def _get_nc():
    if "nc" not in _CACHE:
        _CACHE["nc"] = _build()
    return _CACHE["nc"]


class _Runner:
    """Jit the SPMD bass_exec once; allow repeated calls (for timing)."""

    def __init__(self, nc, donate=True):
        import jax
        from jax.experimental.shard_map import shard_map
        from jax.sharding import Mesh, PartitionSpec

        import concourse.mybir as mybir
        from concourse.bass2jax import (
            _bass_exec_p,
            install_neuronx_cc_hook,
            partition_id_tensor,
        )

        install_neuronx_cc_hook()
        self.nc = nc
        partition_name = (
            nc.partition_id_tensor.name if nc.partition_id_tensor else None
        )

        in_names: list[str] = []
        out_names: list[str] = []
        out_avals: list = []
        for alloc in nc.m.functions[0].allocations:
            if not isinstance(alloc, mybir.MemoryLocationSet):
                continue
            name = alloc.memorylocations[0].name
            if alloc.kind == "ExternalInput":
                if name != partition_name:
                    in_names.append(name)
            elif alloc.kind == "ExternalOutput":
                out_names.append(name)
                out_avals.append(
                    jax.core.ShapedArray(
                        tuple(alloc.tensor_shape), mybir.dt.np(alloc.dtype)
                    )
                )
        self.in_names = list(in_names)
        self.out_names = out_names
        self.out_avals = out_avals
        n_params = len(in_names)
        n_outs = len(out_names)
        all_in_names = in_names + out_names
        if partition_name is not None:
            all_in_names = all_in_names + [partition_name]

        def _body(*args):
            operands = list(args)
            if partition_name is not None:
                operands.append(partition_id_tensor())
            outs = _bass_exec_p.bind(
                *operands,
                out_avals=tuple(out_avals),
                in_names=tuple(all_in_names),
                out_names=tuple(out_names),
                lowering_input_output_aliases=(),
                sim_require_finite=True,
                sim_require_nnan=True,
                nc=nc,
            )
            return tuple(outs)

        devices = jax.devices()[:N_CORES]
        mesh = Mesh(np.asarray(devices), ("core",))
        self.mesh = mesh
        in_specs = (PartitionSpec("core"),) * (n_params + n_outs)
        out_specs = (PartitionSpec("core"),) * n_outs
        self._sharded = jax.jit(
            shard_map(
                _body,
                mesh=mesh,
                in_specs=in_specs,
                out_specs=out_specs,
                check_rep=False,
            ),
            donate_argnums=tuple(range(n_params, n_params + n_outs))
            if donate
            else (),
            keep_unused=True,
        )

    def device_args(self, in_maps):
        """device_put all operands once (inputs + zero out buffers)."""
        import jax
        from jax.sharding import NamedSharding, PartitionSpec

        sh = NamedSharding(self.mesh, PartitionSpec("core"))
        concat_in = [
            np.concatenate([np.asarray(m[name]) for m in in_maps], axis=0)
            for name in self.in_names
        ]
        concat_zeros = [
            np.zeros((N_CORES * a.shape[0], *a.shape[1:]), a.dtype)
            for a in self.out_avals
        ]
        return tuple(jax.device_put(a, sh) for a in (*concat_in, *concat_zeros))

    def run_prepared(self, dev_args):
        import jax

        return jax.block_until_ready(self._sharded(*dev_args))

    def __call__(self, in_maps):
        import jax

        concat_in = [
            np.concatenate([np.asarray(m[name]) for m in in_maps], axis=0)
            for name in self.in_names
        ]
        concat_zeros = [
            np.zeros((N_CORES * a.shape[0], *a.shape[1:]), a.dtype)
            for a in self.out_avals
        ]
        out_arrs = self._sharded(*concat_in, *concat_zeros)
        out_arrs = jax.block_until_ready(out_arrs)
        return [
            {
                name: np.asarray(out_arrs[i]).reshape(
                    N_CORES, *self.out_avals[i].shape
                )[c]
                for i, name in enumerate(self.out_names)
            }
            for c in range(N_CORES)
        ]


def _get_runner():
    if "runner" not in _CACHE:
        _CACHE["runner"] = _Runner(_get_nc())
    return _CACHE["runner"]


def _in_maps(x):
    mvt = _mvt_weights()
    return [{"xs": x[n], "mvt": mvt} for n in range(N_CORES)]


def kernel(x, kernel=None, **_ignored):
    """Full-input entry point: x (8,128,256,256) f32 -> (8,128,128,128) f32."""
    x = np.ascontiguousarray(np.asarray(x, dtype=np.float32))
    assert x.shape == (B, C, H, W), x.shape

    runner = _get_runner()
    in_maps = _in_maps(x)
    try:
        results = runner(in_maps)
    except Exception:
        # One retry for transient device errors (e.g. a wedged NeuronCore
        # recovering); rebuild the jitted callable from scratch.
        _CACHE.pop("runner", None)
        runner = _get_runner()
        results = runner(in_maps)
    outp = np.stack([results[n]["out"] for n in range(N_CORES)], axis=0)
    return outp.astype(np.float32, copy=False)
